# revision 16
# baseline (speedup 1.0000x reference)
"""Trainium2 Bass kernel for nn_MultiHeadAttention3_549755814010.

Math note: in the reference, softmax is taken over the key axis which has
length 1, so the attention weights are identically 1.0 and the whole
l2norm/attention front-end is dead code.  The computation reduces to

    S_b     = sum_d v[b, d]                                  (per-batch scalar)
    z[b,:]  = S_b * v[b,:] + k[b,:]                          (2048, 640)
    y[b,:]  = z[b,:] @ w_fc.T + b_fc                         (small matmul)
    wg[b,:] = y[b,:] * gamma1                                (2048, 640)
    out[b,q,:] = LayerNorm(wg[b,:] + q[b,q,:]) * ln_w + ln_b (the bulk)

The kernel is pure data parallel over the leading num_c=2048 dim across 8
cores (256 batches per core).  Per core the dominant cost is streaming
q (21 MB) in and out (21 MB) -> memory-bound.

Per-core device program:
  prologue: compute z, transpose via PE, matmul against the pre-scaled
            weight (w_fc.T * gamma1, with a K=1 ones-row matmul adding
            b_fc * gamma1) -> wg (256, 640) in SBUF.
  main loop (8 tiles of [128 batches, 8 qpos, 640]):
    - DVE tensor_tensor_reduce: x = q + wg (in place) and s1 = sum(x)
    - ACT Square with accum_out: s2 = sum(x^2)
    - batched per-tile stats: var = s2/D - (s1/D)^2, rstd = 1/sqrt(var+eps)
    - normalize x in place: x * rstd - mean * rstd, spread across
      GPSIMD / DVE / ACT
    - DMA out
"""

import numpy as np
from contextlib import ExitStack

import concourse.bass as bass
import concourse.tile as tile
from concourse import bacc, mybir
from concourse.bass_utils import run_bass_kernel_spmd
from concourse.masks import make_identity

N_CORES = 8
NUM_C, LQ, D = 2048, 32, 640
B = NUM_C // N_CORES          # 256 batches per core
H = B // 128                  # 2 batch halves of 128 (partition dim)
SEG = 8                       # qpos positions per tile
NJ = LQ // SEG                # 4 qpos chunks per batch half
EPS_LN = 1e-5
F32 = mybir.dt.float32
AX = mybir.AxisListType
ALU = mybir.AluOpType
ACTF = mybir.ActivationFunctionType

# engine for the per-segment normalize pass, by segment index
_NORM_ENGINES = ("gpsimd", "gpsimd", "vector", "vector",
                 "scalar", "scalar", "scalar", "scalar")


def _build(ln_trivial: bool) -> bass.Bass:
    # Bacc (not raw Bass): its compile() pipeline runs
    # move_matmul_waits_to_ldweights + generate_event_semaphores, which split
    # multi-sem waits that TRN2 instruction structs cannot encode.
    nc = bacc.Bacc("TRN2", name="mha3_549755814010")

    q = nc.dram_tensor("q", (B, LQ * D), F32, kind="ExternalInput")
    kk = nc.dram_tensor("kk", (B, D), F32, kind="ExternalInput")
    vv = nc.dram_tensor("vv", (B, D), F32, kind="ExternalInput")
    wgw = nc.dram_tensor("wgw", (D, D), F32, kind="ExternalInput")
    wgb = nc.dram_tensor("wgb", (1, D), F32, kind="ExternalInput")
    if not ln_trivial:
        lnw = nc.dram_tensor("lnw", (1, D), F32, kind="ExternalInput")
        lnb = nc.dram_tensor("lnb", (1, D), F32, kind="ExternalInput")
    o = nc.dram_tensor("o", (B, LQ * D), F32, kind="ExternalOutput")

    with ExitStack() as ctx:
        tc = ctx.enter_context(tile.TileContext(nc))
        const = ctx.enter_context(tc.tile_pool(name="const", bufs=1))
        work = ctx.enter_context(tc.tile_pool(name="work", bufs=4))
        qpool = ctx.enter_context(tc.tile_pool(name="qpool", bufs=4))
        stat = ctx.enter_context(tc.tile_pool(name="stat", bufs=4))
        psum_t = ctx.enter_context(tc.tile_pool(name="psum_t", bufs=2, space="PSUM"))
        psum_y = ctx.enter_context(tc.tile_pool(name="psum_y", bufs=2, space="PSUM"))

        # ---- constants ----
        ident = const.tile([128, 128], F32)
        make_identity(nc, ident)
        ones_row = const.tile([1, 128], F32)
        nc.vector.memset(ones_row, 1.0)
        eps_t = const.tile([128, 1], F32)
        nc.vector.memset(eps_t, EPS_LN)

        wgw_sb = const.tile([128, 5, D], F32)
        for c in range(5):
            nc.sync.dma_start(out=wgw_sb[:, c, :], in_=wgw[c * 128:(c + 1) * 128, :])
        wgb_sb = const.tile([1, D], F32)
        nc.sync.dma_start(out=wgb_sb, in_=wgb[:, :])

        kt = const.tile([128, H, D], F32)
        vt = const.tile([128, H, D], F32)
        for h in range(H):
            nc.sync.dma_start(out=kt[:, h, :], in_=kk[h * 128:(h + 1) * 128, :])
            nc.sync.dma_start(out=vt[:, h, :], in_=vv[h * 128:(h + 1) * 128, :])

        if not ln_trivial:
            lnw_b = const.tile([128, D], F32)
            lnb_b = const.tile([128, D], F32)
            nc.sync.dma_start(out=lnw_b, in_=lnw.to_broadcast((128, D)))
            nc.sync.dma_start(out=lnb_b, in_=lnb.to_broadcast((128, D)))

        # ---- prologue ----
        # z = rowsum(v) * v + k, but the "+ k" is folded into the PSUM
        # accumulation of the wg matmul:  wg = (S*v) @ W + k @ W + b_fc*gamma.
        # (Avoids plain TensorTensor instructions — walrus's TT struct can
        # only encode a single sync wait and Tile may attach two.)
        vpT = const.tile([128, 5, B], F32)  # (S*v).T  [i % 128, i // 128, b]
        kT = const.tile([128, 5, B], F32)   # k.T
        for h in range(H):
            s_v = stat.tile([128, 1], F32, tag="s_v")
            nc.vector.reduce_sum(out=s_v, in_=vt[:, h, :], axis=AX.X)
            vp = work.tile([128, D], F32, tag="vp")
            nc.vector.tensor_scalar(out=vp, in0=vt[:, h, :], scalar1=s_v,
                                    scalar2=None, op0=ALU.mult)
            for c in range(5):
                csl = slice(c * 128, (c + 1) * 128)
                pt = psum_t.tile([128, 128], F32)
                nc.tensor.transpose(pt, vp[:, csl], ident)
                nc.scalar.copy(out=vpT[:, c, h * 128:(h + 1) * 128], in_=pt)
                pt2 = psum_t.tile([128, 128], F32)
                nc.tensor.transpose(pt2, kt[:, h, csl], ident)
                nc.scalar.copy(out=kT[:, c, h * 128:(h + 1) * 128], in_=pt2)

        wg = const.tile([128, H, D], F32)   # (y + b_fc) * gamma1
        for h in range(H):
            hsl = slice(h * 128, (h + 1) * 128)
            for oo in range(2):
                osl = slice(oo * 320, (oo + 1) * 320)
                py = psum_y.tile([128, 320], F32)
                for c in range(5):
                    nc.tensor.matmul(py, lhsT=vpT[:, c, hsl],
                                     rhs=wgw_sb[:, c, osl],
                                     start=(c == 0), stop=False)
                for c in range(5):
                    nc.tensor.matmul(py, lhsT=kT[:, c, hsl],
                                     rhs=wgw_sb[:, c, osl],
                                     start=False, stop=False)
                nc.tensor.matmul(py, lhsT=ones_row[:, :],
                                 rhs=wgb_sb[:, osl], start=False, stop=True)
                nc.scalar.copy(out=wg[:, h, osl], in_=py)

        # ---- main loop over q tiles ----
        for h in range(H):
            for j in range(NJ):
                rows = slice(h * 128, (h + 1) * 128)
                cols = slice(j * SEG * D, (j + 1) * SEG * D)
                qt = qpool.tile([128, SEG, D], F32)
                nc.sync.dma_start(out=qt, in_=q[rows, cols].rearrange(
                    "p (s d) -> p s d", s=SEG))

                s1 = stat.tile([128, SEG], F32)
                s2 = stat.tile([128, SEG], F32)
                for s in range(SEG):
                    # x = q + wg, in place; add split across DVE and GPSIMD
                    eng = nc.vector if s % 2 == 0 else nc.gpsimd
                    eng.tensor_add(out=qt[:, s, :], in0=qt[:, s, :],
                                   in1=wg[:, h, :])
                    nc.vector.reduce_sum(out=s1[:, s:s + 1], in_=qt[:, s, :],
                                         axis=AX.X)
                    xsq = work.tile([128, D], F32, tag="xsq")
                    nc.scalar.activation(out=xsq, in_=qt[:, s, :],
                                         func=ACTF.Square,
                                         accum_out=s2[:, s:s + 1])

                # batched stats: negm = -s1/D ; var = s2/D - negm^2
                negm = stat.tile([128, SEG], F32)
                nc.vector.tensor_scalar(out=negm, in0=s1, scalar1=-1.0 / D,
                                        scalar2=None, op0=ALU.mult)
                msq = stat.tile([128, SEG], F32)
                nc.scalar.activation(out=msq, in_=negm, func=ACTF.Square)
                var = stat.tile([128, SEG], F32)
                nc.vector.tensor_scalar(out=var, in0=s2, scalar1=1.0 / D,
                                        scalar2=None, op0=ALU.mult)
                nc.vector.tensor_sub(out=var, in0=var, in1=msq)
                std = stat.tile([128, SEG], F32)
                nc.scalar.activation(out=std, in_=var, func=ACTF.Sqrt,
                                     bias=eps_t, scale=1.0)
                rstd = stat.tile([128, SEG], F32)
                nc.vector.reciprocal(out=rstd, in_=std)
                nmr = stat.tile([128, SEG], F32)   # -mean * rstd
                nc.vector.tensor_mul(out=nmr, in0=negm, in1=rstd)

                for s in range(SEG):
                    eng = _NORM_ENGINES[s]
                    if eng == "scalar":
                        nc.scalar.activation(out=qt[:, s, :], in_=qt[:, s, :],
                                             func=ACTF.Identity,
                                             bias=nmr[:, s:s + 1],
                                             scale=rstd[:, s:s + 1])
                    else:
                        getattr(nc, eng).tensor_scalar(
                            out=qt[:, s, :], in0=qt[:, s, :],
                            scalar1=rstd[:, s:s + 1], scalar2=nmr[:, s:s + 1],
                            op0=ALU.mult, op1=ALU.add)
                    if not ln_trivial:
                        e2 = nc.vector if s % 2 == 0 else nc.gpsimd
                        e2.tensor_mul(out=qt[:, s, :], in0=qt[:, s, :],
                                      in1=lnw_b)
                        e2.tensor_add(out=qt[:, s, :], in0=qt[:, s, :],
                                      in1=lnb_b)

                nc.sync.dma_start(out=o[rows, cols].rearrange(
                    "p (s d) -> p s d", s=SEG), in_=qt)

    nc.finalize()
    return nc


_NC_CACHE: dict = {}


def _prepare(q, k, v, w_fc, b_fc, gamma1, ln_w, ln_b):
    qf = np.ascontiguousarray(np.asarray(q, np.float32)).reshape(NUM_C, LQ * D)
    kf = np.ascontiguousarray(np.asarray(k, np.float32)).reshape(NUM_C, D)
    vf = np.ascontiguousarray(np.asarray(v, np.float32)).reshape(NUM_C, D)
    g = np.asarray(gamma1, np.float32)
    wgw = np.ascontiguousarray(np.asarray(w_fc, np.float32).T * g[None, :])
    wgb = np.ascontiguousarray((np.asarray(b_fc, np.float32) * g).reshape(1, D))
    lnw = np.asarray(ln_w, np.float32)
    lnb = np.asarray(ln_b, np.float32)
    ln_trivial = bool(np.all(lnw == 1.0) and np.all(lnb == 0.0))

    in_maps = []
    for i in range(N_CORES):
        rows = slice(i * B, (i + 1) * B)
        m = {"q": qf[rows], "kk": kf[rows], "vv": vf[rows],
             "wgw": wgw, "wgb": wgb}
        if not ln_trivial:
            m["lnw"] = lnw.reshape(1, D)
            m["lnb"] = lnb.reshape(1, D)
        in_maps.append(m)
    return in_maps, ln_trivial


def _postprocess(results):
    return np.concatenate(
        [r["o"].reshape(B, LQ, D) for r in results], axis=0)


def run(inputs: dict, trace: bool = False, tmpdir=None):
    in_maps, ln_trivial = _prepare(**inputs)
    key = ln_trivial
    if key not in _NC_CACHE:
        _NC_CACHE[key] = _build(ln_trivial)
    nc = _NC_CACHE[key]
    res = run_bass_kernel_spmd(nc, in_maps, core_ids=list(range(N_CORES)),
                               trace=trace, tmpdir=tmpdir)
    return _postprocess(res.results), res


def kernel(**inputs) -> np.ndarray:
    out, _ = run(inputs, trace=False)
    return out
